# revision 1
# baseline (speedup 1.0000x reference)
"""CoAttention kernel for 8x TRN2 NeuronCores.

Computation (per batch b):
    q = x[b] @ Wq.T + bq            [Sq, H]
    k = y[b] @ Wk.T + bk            [Skv, H]
    v = y[b] @ Wv.T + bv            [Skv, H]
    out[b] = softmax(q @ k.T / sqrt(H)) @ v

Sharding: data-parallel over batch; each of the 8 cores handles B/8 = 2
batches. Weights are replicated. Host staging transposes activations to
[D, S] (contraction dim on partitions) and casts matmul operands to fp16
(PE runs fp16 at 4x the fp32 rate; fp32 accumulation in PSUM keeps the
absmax-relative error ~4e-4, verified against a float64 reference).

Device-side layout choices:
  - Q^T [H, Sq] and K^T [H, Skv] (H on partitions) so the score matmul
    contracts over H, and the per-partition bias add is free on DVE.
  - Scores are built TRANSPOSED: S^T[t, s] = (K^T tile).T @ Q^T, so that
    P^T = exp(S^T) is directly usable as the stationary operand of the
    P @ V matmul (contraction over t on partitions).
  - Softmax denominator comes for free as a ones-column appended to V:
    out_psum[:, H] = sum_t P^T[t, s]. No max-subtraction is needed:
    logits are O(1) here, exp cannot overflow, and softmax is shift-
    invariant so the result matches the reference exactly.
  - bv is folded past the softmax: rows of softmax sum to 1, so
    out = (P @ v_raw) / denom + bv.
"""

import os
import sys
from contextlib import ExitStack

import numpy as np

sys.path.insert(0, "/opt/trn_rl_repo")

N_CORES = 8
B, SQ, SKV, D, H = 16, 1024, 1024, 768, 256
BL = B // N_CORES  # batches per core
KD = D // 128      # 6 contraction tiles for the projections
JH = H // 128      # 2 partition tiles of hidden
TS = SKV // 128    # 8 kv tiles
SB = SQ // 512     # 2 query blocks of 512

_cached = {}


def _build_nc(reps=1):
    import concourse.bass as bass
    import concourse.tile as tile
    from concourse import bacc, mybir

    f16 = mybir.dt.float16
    f32 = mybir.dt.float32
    Exp = mybir.ActivationFunctionType.Exp
    Copy = mybir.ActivationFunctionType.Copy
    mult = mybir.AluOpType.mult
    add = mybir.AluOpType.add

    nc = bacc.Bacc("TRN2", target_bir_lowering=False, debug=False)

    xT = nc.dram_tensor("xT", [BL, D, SQ], f16, kind="ExternalInput")
    yT = nc.dram_tensor("yT", [BL, D, SKV], f16, kind="ExternalInput")
    wqT = nc.dram_tensor("wqT", [D, H], f16, kind="ExternalInput")
    wkT = nc.dram_tensor("wkT", [D, H], f16, kind="ExternalInput")
    wvT = nc.dram_tensor("wvT", [D, H], f16, kind="ExternalInput")
    # biases packed host-side into one tensor -> one DMA (HWDGE descriptor
    # generation is ~0.6us per dma_start regardless of size):
    # cols [0:JH]=bq tiles, [JH:2*JH]=bk tiles, [2*JH:2*JH+H]=bv broadcast.
    biasd = nc.dram_tensor("biases", [128, 2 * JH + H], f32, kind="ExternalInput")
    outd = nc.dram_tensor("out", [BL, SQ, H], f32, kind="ExternalOutput")

    with tile.TileContext(nc) as tc, ExitStack() as ctx:
        wpool = ctx.enter_context(tc.tile_pool(name="w", bufs=1))
        cpool = ctx.enter_context(tc.tile_pool(name="c", bufs=1))
        xpool = ctx.enter_context(tc.tile_pool(name="acts", bufs=2))
        qkv = ctx.enter_context(tc.tile_pool(name="qkv", bufs=2))
        ptp = ctx.enter_context(
            tc.tile_pool(name="ptp", bufs=int(os.environ.get("KERNEL_PTP_BUFS", "6")))
        )
        outp = ctx.enter_context(tc.tile_pool(name="outp", bufs=4))
        smallp = ctx.enter_context(tc.tile_pool(name="small", bufs=4))
        psA = ctx.enter_context(
            tc.tile_pool(name="psA", bufs=2, space=bass.MemorySpace.PSUM)
        )
        psS = ctx.enter_context(
            tc.tile_pool(name="psS", bufs=2, space=bass.MemorySpace.PSUM)
        )
        psO = ctx.enter_context(
            tc.tile_pool(name="psO", bufs=4, space=bass.MemorySpace.PSUM)
        )

        # The first real matmul can't start until wq + the first x slices
        # land (~4us of DMA latency). Matmuls issued in the first ~3.4us
        # of PE activity run at half clock (HAM cold / pstate ramp), so
        # burn that window on dummy matmuls over zeroed scratch — by the
        # time real work arrives the PE is at 2.4GHz.
        if int(os.environ.get("KERNEL_WARMUP_MMS", "14")):
            warm_sb = cpool.tile([128, 512], f16, tag="warm")
            nc.vector.memset(warm_sb[:], 0.0)
            warm_ps = psS.tile([128, 512], f32, tag="st", name="warm_ps")
            for _ in range(int(os.environ.get("KERNEL_WARMUP_MMS", "14"))):
                nc.tensor.matmul(
                    warm_ps[:], warm_sb[:, 0:128], warm_sb[:],
                    start=True, stop=True,
                )

        # Replicated constants. Every dma_start pays ~0.6us of serialized
        # HWDGE descriptor generation, so transfers are batched into few
        # large ops, issued in first-needed order: biases+wq (first matmul
        # group), x, wk, y, wv.
        wq_sb = wpool.tile([128, KD, H], f16, tag="wq")
        nc.sync.dma_start(wq_sb[:], wqT[:].rearrange("(k p) h -> p k h", p=128))

        def emit_acts(dram, b, tagp, mid=None):
            # One [128, KD, S] tile per activation tensor, loaded in a few
            # k-chunked ops so matmul groups start at partial arrival.
            nops = int(os.environ.get("KERNEL_ACT_DMAS", "2"))
            t = xpool.tile([128, KD, SQ], f16, tag=tagp, name=f"{tagp}_{b}")
            src = dram[b].rearrange("(k p) s -> p k s", p=128)
            bounds = [KD * i // nops for i in range(nops + 1)]
            for i in range(nops):
                nc.sync.dma_start(
                    t[:, bounds[i] : bounds[i + 1], :],
                    src[:, bounds[i] : bounds[i + 1], :],
                )
                if mid is not None and i == 0:
                    mid()
            return [t[:, k, :] for k in range(KD)]

        xts0 = emit_acts(xT, 0, "xt") if reps == 1 else None
        bias_sb = cpool.tile([128, 2 * JH + H], f32, tag="bias")
        nc.sync.dma_start(bias_sb[:], biasd[:])
        bq_sb = bias_sb[:, 0:JH]
        bk_sb = bias_sb[:, JH : 2 * JH]
        bv_sb = bias_sb[:, 2 * JH : 2 * JH + H]
        wk_sb = wpool.tile([128, KD, H], f16, tag="wk")
        nc.sync.dma_start(wk_sb[:], wkT[:].rearrange("(k p) h -> p k h", p=128))
        wv_sb = wpool.tile([128, KD, H], f16, tag="wv")

        def load_wv():
            nc.sync.dma_start(
                wv_sb[:], wvT[:].rearrange("(k p) h -> p k h", p=128)
            )

        if reps == 1:
            if os.environ.get("KERNEL_WV_EARLY"):
                load_wv()
                yts0 = emit_acts(yT, 0, "yt")
            else:
                yts0 = emit_acts(yT, 0, "yt", mid=load_wv)
        else:
            yts0 = None
            load_wv()

        def emit_body(first=False):
            for b in range(BL):
                if first and b == 0:
                    emit_batch(b, xts0, yts0)
                else:
                    emit_batch(b, emit_acts(xT, b, "xt"), emit_acts(yT, b, "yt"))

        def emit_batch(b, xts, yts):

            qt_sb = qkv.tile([128, JH, SQ], f16, tag="qt", name=f"qt_{b}")
            kt_sb = qkv.tile([128, JH, SKV], f16, tag="kt", name=f"kt_{b}")
            v_sb = qkv.tile([128, TS, H + 1], f16, tag="v", name=f"v_{b}")

            # Q^T / K^T projections: psum[h, s_half] += WxT_k.T @ actT_k
            for w_sb, acts, bias_sb, dst in (
                (wq_sb, xts, bq_sb, qt_sb),
                (wk_sb, yts, bk_sb, kt_sb),
            ):
                for j in range(JH):
                    for hv in range(2):
                        pp = psA.tile([128, 512], f32, tag="proj", name=f"pp{b}")
                        for k in range(KD):
                            nc.tensor.matmul(
                                pp[:],
                                w_sb[:, k, 128 * j : 128 * (j + 1)],
                                acts[k][:, 512 * hv : 512 * (hv + 1)],
                                start=(k == 0),
                                stop=(k == KD - 1),
                            )
                        nc.vector.tensor_scalar_add(
                            dst[:, j, 512 * hv : 512 * (hv + 1)],
                            pp[:],
                            bias_sb[:, j : j + 1],
                        )

            # V projection (no bias; folded into the epilogue): V[t, h]
            for t in range(TS):
                pv = psA.tile([128, H], f32, tag="proj", name=f"pv{b}")
                for k in range(KD):
                    nc.tensor.matmul(
                        pv[:],
                        yts[k][:, 128 * t : 128 * (t + 1)],
                        wv_sb[:, k, :],
                        start=(k == 0),
                        stop=(k == KD - 1),
                    )
                nc.scalar.activation(v_sb[:, t, 0:H], pv[:], Copy)
                nc.vector.memset(v_sb[:, t, H : H + 1], 1.0)

            # Attention: flat software pipeline over (sb, t). The P@V
            # matmuls trail the S^T matmuls by one step — across s-block
            # boundaries too — so PE always has exp-independent work in
            # flight while ACT computes exp(t).
            pts = {}
            ops_by_sb = {}

            def emit_st(sb, t):
                st = psS.tile([128, 512], f32, tag="st", name=f"st{b}")
                for j2 in range(JH):
                    nc.tensor.matmul(
                        st[:],
                        kt_sb[:, j2, 128 * t : 128 * (t + 1)],
                        qt_sb[:, j2, 512 * sb : 512 * (sb + 1)],
                        start=(j2 == 0),
                        stop=(j2 == JH - 1),
                    )
                pt = ptp.tile([128, 512], f16, tag="pt", name=f"pt{b}")
                nc.scalar.activation(pt[:], st[:], Exp, scale=float(H) ** -0.5)
                pts[(sb, t)] = pt

            def emit_epilogue(sb, ops):
                # out = pv / denom + bv. All but the very last block stage
                # into one tile and ship as a single DMA (HWDGE op economy);
                # the last block ships per-j so the tail drain is short.
                last = b == BL - 1 and sb == SB - 1
                ot = outp.tile([128, 4, H], f32, tag="ot", name=f"ot{b}")
                for j in range(4):
                    rec = smallp.tile([128, 1], f32, tag="rec", name=f"rec{b}")
                    nc.vector.reciprocal(rec[:], ops[j][:, H : H + 1])
                    nc.vector.scalar_tensor_tensor(
                        ot[:, j, :], ops[j][:, 0:H], rec[:], bv_sb[:],
                        op0=mult, op1=add,
                    )
                    if last:
                        # spread the tail DMAs over idle DGE issuers so the
                        # final drain isn't serialized on SP.SEQ
                        if os.environ.get("KERNEL_TAIL_ENG", "mixed") == "mixed":
                            eng = (nc.sync, nc.scalar, nc.sync, nc.scalar)[j]
                        else:
                            eng = nc.sync
                        si = 4 * sb + j
                        eng.dma_start(
                            outd[b, 128 * si : 128 * (si + 1), :], ot[:, j, :]
                        )
                if not last:
                    nc.sync.dma_start(
                        outd[b, 512 * sb : 512 * (sb + 1), :].rearrange(
                            "(j p) h -> p j h", p=128
                        ),
                        ot[:],
                    )

            def emit_o(sb, t):
                if t == 0:
                    ops_by_sb[sb] = [
                        psO.tile([128, H + 1], f32, tag="o", name=f"op{b}_{sb}_{j}")
                        for j in range(4)
                    ]
                ops = ops_by_sb[sb]
                for j in range(4):
                    nc.tensor.matmul(
                        ops[j][:],
                        pts[(sb, t)][:, 128 * j : 128 * (j + 1)],
                        v_sb[:, t, :],
                        start=(t == 0),
                        stop=(t == TS - 1),
                    )
                del pts[(sb, t)]
                if t == TS - 1:
                    emit_epilogue(sb, ops)
                    del ops_by_sb[sb]

            depth = int(os.environ.get("KERNEL_ATTN_DEPTH", "2"))
            seq = [(sb, t) for sb in range(SB) for t in range(TS)]
            for i, (sb, t) in enumerate(seq):
                emit_st(sb, t)
                if i >= depth:
                    emit_o(*seq[i - depth])
            for i in range(len(seq) - depth, len(seq)):
                emit_o(*seq[i])

        if reps == 1:
            emit_body(first=True)
        else:
            # Device-side repetition for wall-clock benchmarking (the
            # per-call dispatch overhead through axon is ~80ms, far above
            # the kernel's span; the R-vs-1 slope isolates HW time).
            with tc.For_i(0, reps, 1):
                emit_body()

    nc.compile()
    return nc


def _get_nc(reps=1):
    key = ("nc", reps)
    if key not in _cached:
        _cached[key] = _build_nc(reps)
    return _cached[key]


def make_in_maps(x, y, Wq, bq, Wk, bk, Wv, bv):

    f16 = np.float16
    wq_h = np.ascontiguousarray(Wq.T).astype(f16)  # [D, H]
    wk_h = np.ascontiguousarray(Wk.T).astype(f16)
    wv_h = np.ascontiguousarray(Wv.T).astype(f16)
    bias_h = np.empty((128, 2 * JH + H), np.float32)
    bias_h[:, 0:JH] = np.asarray(bq, np.float32).reshape(JH, 128).T
    bias_h[:, JH : 2 * JH] = np.asarray(bk, np.float32).reshape(JH, 128).T
    bias_h[:, 2 * JH :] = np.asarray(bv, np.float32)[None, :]

    in_maps = []
    for c in range(N_CORES):
        xs = np.asarray(x[BL * c : BL * (c + 1)])  # [BL, Sq, D]
        ys = np.asarray(y[BL * c : BL * (c + 1)])
        in_maps.append(
            {
                "xT": np.ascontiguousarray(xs.transpose(0, 2, 1)).astype(f16),
                "yT": np.ascontiguousarray(ys.transpose(0, 2, 1)).astype(f16),
                "wqT": wq_h,
                "wkT": wk_h,
                "wvT": wv_h,
                "biases": bias_h,
            }
        )
    return in_maps


def kernel(x, y, Wq, bq, Wk, bk, Wv, bv):
    from concourse.bass_utils import run_bass_kernel_spmd

    nc = _get_nc()
    in_maps = make_in_maps(x, y, Wq, bq, Wk, bk, Wv, bv)
    bkr = run_bass_kernel_spmd(
        nc,
        in_maps,
        list(range(N_CORES)),
        trace=bool(os.environ.get("KERNEL_TRACE")),
    )
    _cached["last_results"] = bkr
    return np.concatenate([r["out"] for r in bkr.results], axis=0)



# revision 9
# speedup vs baseline: 1.0525x; 1.0525x over previous
"""CoAttention kernel for 8x TRN2 NeuronCores.

Computation (per batch b):
    q = x[b] @ Wq.T + bq            [Sq, H]
    k = y[b] @ Wk.T + bk            [Skv, H]
    v = y[b] @ Wv.T + bv            [Skv, H]
    out[b] = softmax(q @ k.T / sqrt(H)) @ v

Sharding: data-parallel over batch; each of the 8 cores handles B/8 = 2
batches. Weights are replicated. Host staging transposes activations to
[D, S] (contraction dim on partitions) and casts matmul operands to fp16
(PE runs fp16 at 4x the fp32 rate; fp32 accumulation in PSUM keeps the
absmax-relative error ~4e-4, verified against a float64 reference).

Device-side layout choices:
  - Q^T [H, Sq] and K^T [H, Skv] (H on partitions) so the score matmul
    contracts over H, and the per-partition bias add is free on DVE.
  - Scores are built TRANSPOSED: S^T[t, s] = (K^T tile).T @ Q^T, so that
    P^T = exp(S^T) is directly usable as the stationary operand of the
    P @ V matmul (contraction over t on partitions).
  - Softmax denominator comes for free as a ones-column appended to V:
    out_psum[:, H] = sum_t P^T[t, s]. No max-subtraction is needed:
    logits are O(1) here, exp cannot overflow, and softmax is shift-
    invariant so the result matches the reference exactly.
  - bv is folded past the softmax: rows of softmax sum to 1, so
    out = (P @ v_raw) / denom + bv.
"""

import os
import sys
from contextlib import ExitStack

import numpy as np

sys.path.insert(0, "/opt/trn_rl_repo")

N_CORES = 8
B, SQ, SKV, D, H = 16, 1024, 1024, 768, 256
BL = B // N_CORES  # batches per core
KD = D // 128      # 6 contraction tiles for the projections
JH = H // 128      # 2 partition tiles of hidden
TS = SKV // 128    # 8 kv tiles
SB = SQ // 512     # 2 query blocks of 512

_cached = {}


def _build_nc(reps=1):
    import concourse.bass as bass
    import concourse.tile as tile
    from concourse import bacc, mybir

    f16 = mybir.dt.float16
    f32 = mybir.dt.float32
    Exp = mybir.ActivationFunctionType.Exp
    Copy = mybir.ActivationFunctionType.Copy
    mult = mybir.AluOpType.mult
    add = mybir.AluOpType.add

    nc = bacc.Bacc("TRN2", target_bir_lowering=False, debug=False)

    xT = nc.dram_tensor("xT", [BL, D, SQ], f16, kind="ExternalInput")
    yT = nc.dram_tensor("yT", [BL, D, SKV], f16, kind="ExternalInput")
    wqT = nc.dram_tensor("wqT", [D, H], f16, kind="ExternalInput")
    wkT = nc.dram_tensor("wkT", [D, H], f16, kind="ExternalInput")
    wvT = nc.dram_tensor("wvT", [D, H], f16, kind="ExternalInput")
    # biases packed host-side into one tensor -> one DMA (HWDGE descriptor
    # generation is ~0.6us per dma_start regardless of size):
    # cols [0:JH]=bq tiles, [JH:2*JH]=bk tiles, [2*JH:2*JH+H]=bv broadcast.
    biasd = nc.dram_tensor("biases", [128, 2 * JH + H], f32, kind="ExternalInput")
    # f16 output: halves the output DMA bytes (tail drain + mid-kernel DMA
    # contention); host upcasts on gather. Adds ~2e-4 rel err (under budget).
    outd = nc.dram_tensor("out", [BL, SQ, H], f16, kind="ExternalOutput")

    with tile.TileContext(nc) as tc, ExitStack() as ctx:
        wpool = ctx.enter_context(tc.tile_pool(name="w", bufs=1))
        cpool = ctx.enter_context(tc.tile_pool(name="c", bufs=1))
        xpool = ctx.enter_context(tc.tile_pool(name="acts", bufs=2))
        qkv = ctx.enter_context(tc.tile_pool(name="qkv", bufs=2))
        ptp = ctx.enter_context(
            tc.tile_pool(name="ptp", bufs=int(os.environ.get("KERNEL_PTP_BUFS", "11")))
        )
        outp = ctx.enter_context(tc.tile_pool(name="outp", bufs=4))
        smallp = ctx.enter_context(tc.tile_pool(name="small", bufs=4))
        psA = ctx.enter_context(
            tc.tile_pool(name="psA", bufs=2, space=bass.MemorySpace.PSUM)
        )
        psS = ctx.enter_context(
            tc.tile_pool(name="psS", bufs=2, space=bass.MemorySpace.PSUM)
        )
        psO = ctx.enter_context(
            tc.tile_pool(name="psO", bufs=4, space=bass.MemorySpace.PSUM)
        )

        # The first real matmul can't start until wq + the first x slices
        # land (~4us of DMA latency). Matmuls issued in the first ~3.4us
        # of PE activity run at half clock (HAM cold / pstate ramp), so
        # burn that window on dummy matmuls over zeroed scratch — by the
        # time real work arrives the PE is at 2.4GHz.
        n_warm = int(os.environ.get("KERNEL_WARMUP_MMS", "14"))
        warm_sb = None
        if n_warm:
            warm_sb = cpool.tile([128, 512], f16, tag="warm")
            nc.vector.memset(warm_sb[:], 0.0)

        def emit_warmup():
            warm_ps = psS.tile([128, 512], f32, tag="st", name="warm_ps")
            for _ in range(n_warm):
                nc.tensor.matmul(
                    warm_ps[:], warm_sb[:, 0:128], warm_sb[:],
                    start=True, stop=True,
                )

        if n_warm and reps == 1:
            emit_warmup()

        # Replicated constants. Every dma_start pays ~0.6us of serialized
        # HWDGE descriptor generation, so transfers are batched into few
        # large ops, issued in first-needed order: biases+wq (first matmul
        # group), x, wk, y, wv.
        wq_sb = wpool.tile([128, KD, H], f16, tag="wq")
        nc.sync.dma_start(wq_sb[:], wqT[:].rearrange("(k p) h -> p k h", p=128))

        def emit_acts(dram, b, tagp, mid=None):
            # One [128, KD, S] tile per activation tensor, loaded in a few
            # k-chunked ops so matmul groups start at partial arrival.
            nops = int(os.environ.get("KERNEL_ACT_DMAS", "2"))
            t = xpool.tile([128, KD, SQ], f16, tag=tagp, name=f"{tagp}_{b}")
            src = dram[b].rearrange("(k p) s -> p k s", p=128)
            bounds = [KD * i // nops for i in range(nops + 1)]
            for i in range(nops):
                nc.sync.dma_start(
                    t[:, bounds[i] : bounds[i + 1], :],
                    src[:, bounds[i] : bounds[i + 1], :],
                )
                if mid is not None and i == 0:
                    mid()
            return [t[:, k, :] for k in range(KD)]

        xts0 = emit_acts(xT, 0, "xt") if reps == 1 else None
        bias_sb = cpool.tile([128, 2 * JH + H], f32, tag="bias")
        nc.sync.dma_start(bias_sb[:], biasd[:])
        bq_sb = bias_sb[:, 0:JH]
        bk_sb = bias_sb[:, JH : 2 * JH]
        bv_sb = bias_sb[:, 2 * JH : 2 * JH + H]
        wk_sb = wpool.tile([128, KD, H], f16, tag="wk")
        nc.sync.dma_start(wk_sb[:], wkT[:].rearrange("(k p) h -> p k h", p=128))
        wv_sb = wpool.tile([128, KD, H], f16, tag="wv")

        def load_wv():
            nc.sync.dma_start(
                wv_sb[:], wvT[:].rearrange("(k p) h -> p k h", p=128)
            )

        if reps == 1:
            if os.environ.get("KERNEL_WV_EARLY"):
                load_wv()
                yts0 = emit_acts(yT, 0, "yt")
            else:
                yts0 = emit_acts(yT, 0, "yt", mid=load_wv)
        else:
            yts0 = None
            load_wv()

        def emit_body(first=False):
            for b in range(BL):
                if first and b == 0:
                    emit_batch(b, xts0, yts0)
                else:
                    emit_batch(b, emit_acts(xT, b, "xt"), emit_acts(yT, b, "yt"))

        def emit_batch(b, xts, yts):

            qt_sb = qkv.tile([128, JH, SQ], f16, tag="qt", name=f"qt_{b}")
            kt_sb = qkv.tile([128, JH, SKV], f16, tag="kt", name=f"kt_{b}")
            v_sb = qkv.tile([128, TS, H + 1], f16, tag="v", name=f"v_{b}")

            # Q^T / K^T projections: psum[h, s_half] += WxT_k.T @ actT_k
            for w_sb, acts, bias_sb, dst in (
                (wq_sb, xts, bq_sb, qt_sb),
                (wk_sb, yts, bk_sb, kt_sb),
            ):
                for j in range(JH):
                    for hv in range(2):
                        pp = psA.tile([128, 512], f32, tag="proj", name=f"pp{b}")
                        for k in range(KD):
                            nc.tensor.matmul(
                                pp[:],
                                w_sb[:, k, 128 * j : 128 * (j + 1)],
                                acts[k][:, 512 * hv : 512 * (hv + 1)],
                                start=(k == 0),
                                stop=(k == KD - 1),
                            )
                        nc.vector.tensor_scalar_add(
                            dst[:, j, 512 * hv : 512 * (hv + 1)],
                            pp[:],
                            bias_sb[:, j : j + 1],
                        )

            # V projection (no bias; folded into the epilogue): V[t, h]
            for t in range(TS):
                pv = psA.tile([128, H], f32, tag="proj", name=f"pv{b}")
                for k in range(KD):
                    nc.tensor.matmul(
                        pv[:],
                        yts[k][:, 128 * t : 128 * (t + 1)],
                        wv_sb[:, k, :],
                        start=(k == 0),
                        stop=(k == KD - 1),
                    )
                nc.scalar.activation(v_sb[:, t, 0:H], pv[:], Copy)
                nc.vector.memset(v_sb[:, t, H : H + 1], 1.0)

            # Attention: flat software pipeline over (sb, t). The P@V
            # matmuls trail the S^T matmuls by one step — across s-block
            # boundaries too — so PE always has exp-independent work in
            # flight while ACT computes exp(t).
            pts = {}
            ops_by_sb = {}

            def emit_st(sb, t):
                st = psS.tile([128, 512], f32, tag="st", name=f"st{b}")
                for j2 in range(JH):
                    nc.tensor.matmul(
                        st[:],
                        kt_sb[:, j2, 128 * t : 128 * (t + 1)],
                        qt_sb[:, j2, 512 * sb : 512 * (sb + 1)],
                        start=(j2 == 0),
                        stop=(j2 == JH - 1),
                    )
                pt = ptp.tile([128, 512], f16, tag="pt", name=f"pt{b}")
                nc.scalar.activation(pt[:], st[:], Exp, scale=float(H) ** -0.5)
                pts[(sb, t)] = pt

            def emit_epilogue(sb, ops):
                # out = pv / denom + bv, staged into one f16 tile and shipped
                # as a single DMA (HWDGE op economy).
                ot = outp.tile([128, 4, H], f16, tag="ot", name=f"ot{b}")
                for j in range(4):
                    rec = smallp.tile([128, 1], f32, tag="rec", name=f"rec{b}")
                    nc.vector.reciprocal(rec[:], ops[j][:, H : H + 1])
                    nc.vector.scalar_tensor_tensor(
                        ot[:, j, :], ops[j][:, 0:H], rec[:], bv_sb[:],
                        op0=mult, op1=add,
                    )
                nc.sync.dma_start(
                    outd[b, 512 * sb : 512 * (sb + 1), :].rearrange(
                        "(j p) h -> p j h", p=128
                    ),
                    ot[:],
                )

            def emit_o(sb, t):
                if t == 0:
                    ops_by_sb[sb] = [
                        psO.tile([128, H + 1], f32, tag="o", name=f"op{b}_{sb}_{j}")
                        for j in range(4)
                    ]
                ops = ops_by_sb[sb]
                for j in range(4):
                    nc.tensor.matmul(
                        ops[j][:],
                        pts[(sb, t)][:, 128 * j : 128 * (j + 1)],
                        v_sb[:, t, :],
                        start=(t == 0),
                        stop=(t == TS - 1),
                    )
                del pts[(sb, t)]
                if t == TS - 1:
                    emit_epilogue(sb, ops)
                    del ops_by_sb[sb]

            depth = int(os.environ.get("KERNEL_ATTN_DEPTH", "2"))
            seq = [(sb, t) for sb in range(SB) for t in range(TS)]
            if b < BL - 1:
                for i, (sb, t) in enumerate(seq):
                    emit_st(sb, t)
                    if i >= depth:
                        emit_o(*seq[i - depth])
                for i in range(len(seq) - depth, len(seq)):
                    emit_o(*seq[i])
            else:
                # Last batch: the final s-block's P@V runs j-OUTER so each
                # 128-row output strip finishes (and ships) while the next
                # strip's matmuls still occupy PE — only strip j=3's
                # reciprocal+scale+64KB DMA is an exposed tail (~1.2us vs
                # ~5.3us for the block-at-once epilogue).
                for i, (sb, t) in enumerate(seq):
                    emit_st(sb, t)
                    if i >= depth and seq[i - depth][0] < SB - 1:
                        emit_o(*seq[i - depth])
                for i in range(len(seq) - depth, len(seq)):
                    if seq[i][0] < SB - 1:
                        emit_o(*seq[i])
                # HWDGE descriptor generation is ~0.6us per dma_start and
                # serialized, so the tail ships as ONE staged DMA: strips
                # j0-j2's reciprocal+scale land while PE still runs j1-j3's
                # matmuls, leaving only j3's epilogue + one descriptor gen
                # exposed.
                sb = SB - 1
                ot = outp.tile([128, 4, H], f16, tag="ot", name="otlast")
                for j in range(4):
                    op = psO.tile([128, H + 1], f32, tag="o", name=f"olast{j}")
                    for t in range(TS):
                        nc.tensor.matmul(
                            op[:],
                            pts[(sb, t)][:, 128 * j : 128 * (j + 1)],
                            v_sb[:, t, :],
                            start=(t == 0),
                            stop=(t == TS - 1),
                        )
                    rec = smallp.tile([128, 1], f32, tag="rec", name=f"recl{j}")
                    nc.vector.reciprocal(rec[:], op[:, H : H + 1])
                    nc.vector.scalar_tensor_tensor(
                        ot[:, j, :], op[:, 0:H], rec[:], bv_sb[:],
                        op0=mult, op1=add,
                    )
                nc.sync.dma_start(
                    outd[b, 512 * sb : 512 * (sb + 1), :].rearrange(
                        "(j p) h -> p j h", p=128
                    ),
                    ot[:],
                )
                for t in range(TS):
                    del pts[(sb, t)]

        if reps == 1:
            emit_body(first=True)
        else:
            # Device-side repetition for wall-clock benchmarking (the
            # per-call dispatch overhead through axon is ~80ms, far above
            # the kernel's span; the R-vs-1 slope isolates HW time).
            # hint_engines: the body is far over 256 instructions per
            # engine, so without branch-prefetch hints the back-edge
            # misses IRAM and stalls ~3-4us per iteration — measurement
            # overhead, not kernel time.
            if os.environ.get("KERNEL_LOOP_HINTS", "1") == "1":
                hints = tuple(
                    mybir.EngineType[e]
                    for e in ("PE", "Activation", "DVE", "SP", "Pool")
                )
            else:
                hints = ()
            with tc.For_i(0, reps, 1, hint_engines=hints):
                if n_warm:
                    # warm_sb was memset before the loop; per-iteration
                    # warmup only re-runs the matmuls (no DVE dependency
                    # at the head of each iteration).
                    emit_warmup()
                emit_body()

    nc.compile()
    return nc


def _get_nc(reps=1):
    key = ("nc", reps)
    if key not in _cached:
        _cached[key] = _build_nc(reps)
    return _cached[key]


def make_in_maps(x, y, Wq, bq, Wk, bk, Wv, bv):

    f16 = np.float16
    wq_h = np.ascontiguousarray(Wq.T).astype(f16)  # [D, H]
    wk_h = np.ascontiguousarray(Wk.T).astype(f16)
    wv_h = np.ascontiguousarray(Wv.T).astype(f16)
    bias_h = np.empty((128, 2 * JH + H), np.float32)
    bias_h[:, 0:JH] = np.asarray(bq, np.float32).reshape(JH, 128).T
    bias_h[:, JH : 2 * JH] = np.asarray(bk, np.float32).reshape(JH, 128).T
    bias_h[:, 2 * JH :] = np.asarray(bv, np.float32)[None, :]

    in_maps = []
    for c in range(N_CORES):
        xs = np.asarray(x[BL * c : BL * (c + 1)])  # [BL, Sq, D]
        ys = np.asarray(y[BL * c : BL * (c + 1)])
        in_maps.append(
            {
                "xT": np.ascontiguousarray(xs.transpose(0, 2, 1)).astype(f16),
                "yT": np.ascontiguousarray(ys.transpose(0, 2, 1)).astype(f16),
                "wqT": wq_h,
                "wkT": wk_h,
                "wvT": wv_h,
                "biases": bias_h,
            }
        )
    return in_maps


def kernel(x, y, Wq, bq, Wk, bk, Wv, bv):
    from concourse.bass_utils import run_bass_kernel_spmd

    nc = _get_nc()
    in_maps = make_in_maps(x, y, Wq, bq, Wk, bk, Wv, bv)
    bkr = run_bass_kernel_spmd(
        nc,
        in_maps,
        list(range(N_CORES)),
        trace=bool(os.environ.get("KERNEL_TRACE")),
    )
    _cached["last_results"] = bkr
    return np.concatenate(
        [r["out"].astype(np.float32) for r in bkr.results], axis=0
    )



# revision 13
# speedup vs baseline: 1.0624x; 1.0094x over previous
"""CoAttention kernel for 8x TRN2 NeuronCores.

Computation (per batch b):
    q = x[b] @ Wq.T + bq            [Sq, H]
    k = y[b] @ Wk.T + bk            [Skv, H]
    v = y[b] @ Wv.T + bv            [Skv, H]
    out[b] = softmax(q @ k.T / sqrt(H)) @ v

Sharding: data-parallel over batch; each of the 8 cores handles B/8 = 2
batches. Weights are replicated. Host staging transposes activations to
[D, S] (contraction dim on partitions) and casts matmul operands to fp16
(PE runs fp16 at 4x the fp32 rate; fp32 accumulation in PSUM keeps the
absmax-relative error ~4e-4, verified against a float64 reference).

Device-side layout choices:
  - Q^T [H, Sq] and K^T [H, Skv] (H on partitions) so the score matmul
    contracts over H, and the per-partition bias add is free on DVE.
  - Scores are built TRANSPOSED: S^T[t, s] = (K^T tile).T @ Q^T, so that
    P^T = exp(S^T) is directly usable as the stationary operand of the
    P @ V matmul (contraction over t on partitions).
  - Softmax denominator comes for free as a ones-column appended to V:
    out_psum[:, H] = sum_t P^T[t, s]. No max-subtraction is needed:
    logits are O(1) here, exp cannot overflow, and softmax is shift-
    invariant so the result matches the reference exactly.
  - bv is folded past the softmax: rows of softmax sum to 1, so
    out = (P @ v_raw) / denom + bv.
"""

import os
import sys
from contextlib import ExitStack

import numpy as np

sys.path.insert(0, "/opt/trn_rl_repo")

N_CORES = 8
B, SQ, SKV, D, H = 16, 1024, 1024, 768, 256
BL = B // N_CORES  # batches per core
KD = D // 128      # 6 contraction tiles for the projections
JH = H // 128      # 2 partition tiles of hidden
TS = SKV // 128    # 8 kv tiles
SB = SQ // 512     # 2 query blocks of 512

_cached = {}


def _build_nc(reps=1):
    import concourse.bass as bass
    import concourse.tile as tile
    from concourse import bacc, mybir

    f16 = mybir.dt.float16
    f32 = mybir.dt.float32
    Exp = mybir.ActivationFunctionType.Exp
    Copy = mybir.ActivationFunctionType.Copy
    mult = mybir.AluOpType.mult
    add = mybir.AluOpType.add

    nc = bacc.Bacc("TRN2", target_bir_lowering=False, debug=False)

    xT = nc.dram_tensor("xT", [BL, D, SQ], f16, kind="ExternalInput")
    yT = nc.dram_tensor("yT", [BL, D, SKV], f16, kind="ExternalInput")
    wqT = nc.dram_tensor("wqT", [D, H], f16, kind="ExternalInput")
    wkT = nc.dram_tensor("wkT", [D, H], f16, kind="ExternalInput")
    wvT = nc.dram_tensor("wvT", [D, H], f16, kind="ExternalInput")
    # biases packed host-side into one tensor -> one DMA (HWDGE descriptor
    # generation is ~0.6us per dma_start regardless of size):
    # cols [0:JH]=bq tiles, [JH:2*JH]=bk tiles, [2*JH:2*JH+H]=bv broadcast.
    biasd = nc.dram_tensor("biases", [128, 2 * JH + H], f32, kind="ExternalInput")
    # f16 output: halves the output DMA bytes (tail drain + mid-kernel DMA
    # contention); host upcasts on gather. Adds ~2e-4 rel err (under budget).
    outd = nc.dram_tensor("out", [BL, SQ, H], f16, kind="ExternalOutput")

    with tile.TileContext(nc) as tc, ExitStack() as ctx:
        wpool = ctx.enter_context(tc.tile_pool(name="w", bufs=1))
        cpool = ctx.enter_context(tc.tile_pool(name="c", bufs=1))
        xpool = ctx.enter_context(tc.tile_pool(name="acts", bufs=2))
        qkv = ctx.enter_context(tc.tile_pool(name="qkv", bufs=2))
        ptp = ctx.enter_context(
            tc.tile_pool(name="ptp", bufs=int(os.environ.get("KERNEL_PTP_BUFS", "11")))
        )
        outp = ctx.enter_context(tc.tile_pool(name="outp", bufs=4))
        smallp = ctx.enter_context(tc.tile_pool(name="small", bufs=4))
        psA = ctx.enter_context(
            tc.tile_pool(name="psA", bufs=2, space=bass.MemorySpace.PSUM)
        )
        psS = ctx.enter_context(
            tc.tile_pool(name="psS", bufs=2, space=bass.MemorySpace.PSUM)
        )
        psO = ctx.enter_context(
            tc.tile_pool(name="psO", bufs=4, space=bass.MemorySpace.PSUM)
        )

        # The first real matmul can't start until wq + the first x slices
        # land (~4us of DMA latency). Matmuls issued in the first ~3.4us
        # of PE activity run at half clock (HAM cold / pstate ramp), so
        # burn that window on dummy matmuls over zeroed scratch — by the
        # time real work arrives the PE is at 2.4GHz.
        n_warm = int(os.environ.get("KERNEL_WARMUP_MMS", "14"))
        n_warm_loop = int(os.environ.get("KERNEL_LOOP_WARMUP_MMS", str(n_warm)))
        warm_sb = None
        if n_warm or (n_warm_loop and reps > 1):
            warm_sb = cpool.tile([128, 512], f16, tag="warm")
            nc.vector.memset(warm_sb[:], 0.0)

        def emit_warmup(n):
            warm_ps = psS.tile([128, 512], f32, tag="st", name="warm_ps")
            for _ in range(n):
                nc.tensor.matmul(
                    warm_ps[:], warm_sb[:, 0:128], warm_sb[:],
                    start=True, stop=True,
                )

        if n_warm and reps == 1:
            emit_warmup(n_warm)

        # Replicated constants. Every dma_start pays ~0.6us of serialized
        # HWDGE descriptor generation, so transfers are batched into few
        # large ops, issued in first-needed order: biases+wq (first matmul
        # group), x, wk, y, wv.
        wq_sb = wpool.tile([128, KD, H], f16, tag="wq")
        nc.sync.dma_start(wq_sb[:], wqT[:].rearrange("(k p) h -> p k h", p=128))

        def emit_acts(dram, b, tagp, mid=None):
            # One [128, KD, S] tile per activation tensor, loaded in a few
            # k-chunked ops so matmul groups start at partial arrival.
            nops = int(os.environ.get("KERNEL_ACT_DMAS", "2"))
            t = xpool.tile([128, KD, SQ], f16, tag=tagp, name=f"{tagp}_{b}")
            src = dram[b].rearrange("(k p) s -> p k s", p=128)
            bounds = [KD * i // nops for i in range(nops + 1)]
            for i in range(nops):
                nc.sync.dma_start(
                    t[:, bounds[i] : bounds[i + 1], :],
                    src[:, bounds[i] : bounds[i + 1], :],
                )
                if mid is not None and i == 0:
                    mid()
            return [t[:, k, :] for k in range(KD)]

        xts0 = emit_acts(xT, 0, "xt") if reps == 1 else None
        bias_sb = cpool.tile([128, 2 * JH + H], f32, tag="bias")
        nc.sync.dma_start(bias_sb[:], biasd[:])
        bq_sb = bias_sb[:, 0:JH]
        bk_sb = bias_sb[:, JH : 2 * JH]
        bv_sb = bias_sb[:, 2 * JH : 2 * JH + H]
        wk_sb = wpool.tile([128, KD, H], f16, tag="wk")
        nc.sync.dma_start(wk_sb[:], wkT[:].rearrange("(k p) h -> p k h", p=128))
        wv_sb = wpool.tile([128, KD, H], f16, tag="wv")

        def load_wv():
            nc.sync.dma_start(
                wv_sb[:], wvT[:].rearrange("(k p) h -> p k h", p=128)
            )

        if reps == 1:
            if os.environ.get("KERNEL_WV_EARLY"):
                load_wv()
                yts0 = emit_acts(yT, 0, "yt")
            else:
                yts0 = emit_acts(yT, 0, "yt", mid=load_wv)
        else:
            yts0 = None
            load_wv()

        def emit_body(first=False):
            for b in range(BL):
                if first and b == 0:
                    emit_batch(b, xts0, yts0)
                else:
                    emit_batch(b, emit_acts(xT, b, "xt"), emit_acts(yT, b, "yt"))

        def emit_batch(b, xts, yts):

            qt_sb = qkv.tile([128, JH, SQ], f16, tag="qt", name=f"qt_{b}")
            kt_sb = qkv.tile([128, JH, SKV], f16, tag="kt", name=f"kt_{b}")
            v_sb = qkv.tile([128, TS, H + 1], f16, tag="v", name=f"v_{b}")

            # Q^T / K^T projections: psum[h, s_half] += WxT_k.T @ actT_k
            for w_sb, acts, bias_sb, dst in (
                (wq_sb, xts, bq_sb, qt_sb),
                (wk_sb, yts, bk_sb, kt_sb),
            ):
                for j in range(JH):
                    for hv in range(2):
                        pp = psA.tile([128, 512], f32, tag="proj", name=f"pp{b}")
                        for k in range(KD):
                            nc.tensor.matmul(
                                pp[:],
                                w_sb[:, k, 128 * j : 128 * (j + 1)],
                                acts[k][:, 512 * hv : 512 * (hv + 1)],
                                start=(k == 0),
                                stop=(k == KD - 1),
                            )
                        nc.vector.tensor_scalar_add(
                            dst[:, j, 512 * hv : 512 * (hv + 1)],
                            pp[:],
                            bias_sb[:, j : j + 1],
                        )

            # V projection (no bias; folded into the epilogue): V[t, h]
            for t in range(TS):
                pv = psA.tile([128, H], f32, tag="proj", name=f"pv{b}")
                for k in range(KD):
                    nc.tensor.matmul(
                        pv[:],
                        yts[k][:, 128 * t : 128 * (t + 1)],
                        wv_sb[:, k, :],
                        start=(k == 0),
                        stop=(k == KD - 1),
                    )
                nc.scalar.activation(v_sb[:, t, 0:H], pv[:], Copy)
                nc.vector.memset(v_sb[:, t, H : H + 1], 1.0)

            # Attention: flat software pipeline over (sb, t). The P@V
            # matmuls trail the S^T matmuls by one step — across s-block
            # boundaries too — so PE always has exp-independent work in
            # flight while ACT computes exp(t).
            pts = {}
            ops_by_sb = {}

            def emit_st(sb, t):
                st = psS.tile([128, 512], f32, tag="st", name=f"st{b}")
                for j2 in range(JH):
                    nc.tensor.matmul(
                        st[:],
                        kt_sb[:, j2, 128 * t : 128 * (t + 1)],
                        qt_sb[:, j2, 512 * sb : 512 * (sb + 1)],
                        start=(j2 == 0),
                        stop=(j2 == JH - 1),
                    )
                pt = ptp.tile([128, 512], f16, tag="pt", name=f"pt{b}")
                nc.scalar.activation(pt[:], st[:], Exp, scale=float(H) ** -0.5)
                pts[(sb, t)] = pt

            def emit_epilogue(sb, ops):
                # out = pv / denom + bv, staged into one f16 tile and shipped
                # as a single DMA (HWDGE op economy).
                ot = outp.tile([128, 4, H], f16, tag="ot", name=f"ot{b}")
                for j in range(4):
                    rec = smallp.tile([128, 1], f32, tag="rec", name=f"rec{b}")
                    nc.vector.reciprocal(rec[:], ops[j][:, H : H + 1])
                    nc.vector.scalar_tensor_tensor(
                        ot[:, j, :], ops[j][:, 0:H], rec[:], bv_sb[:],
                        op0=mult, op1=add,
                    )
                nc.sync.dma_start(
                    outd[b, 512 * sb : 512 * (sb + 1), :].rearrange(
                        "(j p) h -> p j h", p=128
                    ),
                    ot[:],
                )

            def emit_o(sb, t):
                if t == 0:
                    ops_by_sb[sb] = [
                        psO.tile([128, H + 1], f32, tag="o", name=f"op{b}_{sb}_{j}")
                        for j in range(4)
                    ]
                ops = ops_by_sb[sb]
                for j in range(4):
                    nc.tensor.matmul(
                        ops[j][:],
                        pts[(sb, t)][:, 128 * j : 128 * (j + 1)],
                        v_sb[:, t, :],
                        start=(t == 0),
                        stop=(t == TS - 1),
                    )
                del pts[(sb, t)]
                if t == TS - 1:
                    emit_epilogue(sb, ops)
                    del ops_by_sb[sb]

            depth = int(os.environ.get("KERNEL_ATTN_DEPTH", "2"))
            seq = [(sb, t) for sb in range(SB) for t in range(TS)]
            if b < BL - 1:
                for i, (sb, t) in enumerate(seq):
                    emit_st(sb, t)
                    if i >= depth:
                        emit_o(*seq[i - depth])
                for i in range(len(seq) - depth, len(seq)):
                    emit_o(*seq[i])
            else:
                # Last batch: the final s-block's P@V runs j-OUTER so each
                # 128-row output strip finishes (and ships) while the next
                # strip's matmuls still occupy PE — only strip j=3's
                # reciprocal+scale+64KB DMA is an exposed tail (~1.2us vs
                # ~5.3us for the block-at-once epilogue).
                for i, (sb, t) in enumerate(seq):
                    emit_st(sb, t)
                    if i >= depth and seq[i - depth][0] < SB - 1:
                        emit_o(*seq[i - depth])
                for i in range(len(seq) - depth, len(seq)):
                    if seq[i][0] < SB - 1:
                        emit_o(*seq[i])
                # HWDGE descriptor generation is ~0.6us per dma_start and
                # serialized, so the tail ships as ONE staged DMA: strips
                # j0-j2's reciprocal+scale land while PE still runs j1-j3's
                # matmuls, leaving only j3's epilogue + one descriptor gen
                # exposed.
                sb = SB - 1
                ot = outp.tile([128, 4, H], f16, tag="ot", name="otlast")
                for j in range(4):
                    op = psO.tile([128, H + 1], f32, tag="o", name=f"olast{j}")
                    for t in range(TS):
                        nc.tensor.matmul(
                            op[:],
                            pts[(sb, t)][:, 128 * j : 128 * (j + 1)],
                            v_sb[:, t, :],
                            start=(t == 0),
                            stop=(t == TS - 1),
                        )
                    rec = smallp.tile([128, 1], f32, tag="rec", name=f"recl{j}")
                    nc.vector.reciprocal(rec[:], op[:, H : H + 1])
                    nc.vector.scalar_tensor_tensor(
                        ot[:, j, :], op[:, 0:H], rec[:], bv_sb[:],
                        op0=mult, op1=add,
                    )
                nc.sync.dma_start(
                    outd[b, 512 * sb : 512 * (sb + 1), :].rearrange(
                        "(j p) h -> p j h", p=128
                    ),
                    ot[:],
                )
                for t in range(TS):
                    del pts[(sb, t)]

        if reps == 1:
            emit_body(first=True)
        else:
            # Device-side repetition for wall-clock benchmarking (the
            # per-call dispatch overhead through axon is ~80ms, far above
            # the kernel's span; the R-vs-1 slope isolates HW time).
            # hint_engines: the body is far over 256 instructions per
            # engine, so without branch-prefetch hints the back-edge
            # misses IRAM and stalls ~3-4us per iteration — measurement
            # overhead, not kernel time.
            if os.environ.get("KERNEL_LOOP_HINTS", "1") == "1":
                hints = tuple(
                    mybir.EngineType[e]
                    for e in ("PE", "Activation", "DVE", "SP", "Pool")
                )
            else:
                hints = ()
            stag = os.environ.get("KERNEL_STAGGER", "1") == "1"
            with tc.For_i(0, reps, 1, hint_engines=hints, staggered_reset=stag):
                if n_warm_loop:
                    # warm_sb was memset before the loop; per-iteration
                    # warmup only re-runs the matmuls (no DVE dependency
                    # at the head of each iteration).
                    emit_warmup(n_warm_loop)
                emit_body()

    nc.compile()
    return nc


def _get_nc(reps=1):
    key = ("nc", reps)
    if key not in _cached:
        _cached[key] = _build_nc(reps)
    return _cached[key]


def make_in_maps(x, y, Wq, bq, Wk, bk, Wv, bv):

    f16 = np.float16
    wq_h = np.ascontiguousarray(Wq.T).astype(f16)  # [D, H]
    wk_h = np.ascontiguousarray(Wk.T).astype(f16)
    wv_h = np.ascontiguousarray(Wv.T).astype(f16)
    bias_h = np.empty((128, 2 * JH + H), np.float32)
    bias_h[:, 0:JH] = np.asarray(bq, np.float32).reshape(JH, 128).T
    bias_h[:, JH : 2 * JH] = np.asarray(bk, np.float32).reshape(JH, 128).T
    bias_h[:, 2 * JH :] = np.asarray(bv, np.float32)[None, :]

    in_maps = []
    for c in range(N_CORES):
        xs = np.asarray(x[BL * c : BL * (c + 1)])  # [BL, Sq, D]
        ys = np.asarray(y[BL * c : BL * (c + 1)])
        in_maps.append(
            {
                "xT": np.ascontiguousarray(xs.transpose(0, 2, 1)).astype(f16),
                "yT": np.ascontiguousarray(ys.transpose(0, 2, 1)).astype(f16),
                "wqT": wq_h,
                "wkT": wk_h,
                "wvT": wv_h,
                "biases": bias_h,
            }
        )
    return in_maps


def kernel(x, y, Wq, bq, Wk, bk, Wv, bv):
    from concourse.bass_utils import run_bass_kernel_spmd

    nc = _get_nc()
    in_maps = make_in_maps(x, y, Wq, bq, Wk, bk, Wv, bv)
    bkr = run_bass_kernel_spmd(
        nc,
        in_maps,
        list(range(N_CORES)),
        trace=bool(os.environ.get("KERNEL_TRACE")),
    )
    _cached["last_results"] = bkr
    return np.concatenate(
        [r["out"].astype(np.float32) for r in bkr.results], axis=0
    )

